# revision 4
# baseline (speedup 1.0000x reference)
"""Cost-volume kernel for Trainium2 (Bass/Tile), 8-core SPMD.

volume[n, c, d, h, w] = left[n,c,h,w] * right[n,c,h,w-d]  (0 where w < d)

Sharding: rows (flattened n,c,h = 8704) split as 1088 per core; every core
computes ALL 48 disparities for its rows. The shift is along W, so row
sharding needs no halo and inputs are read exactly once globally.

The kernel is HBM-store bound (the cost volume is ~100M elements), so the
store stream is minimized two ways:
 - fp16 output (harness gate is rel_err < 2e-2; fp16 product error ~7e-4).
   Inputs are converted to fp16 on host; output is upcast on host.
 - packed layout: the w < d zero wedge (9.8% of the volume) is never
   computed or stored; for disparity d only the W-d valid products
   out[d][r, j] = left[r, d+j] * right[r, j] are written. The host
   scatters them into a zero-filled full volume.

Per core: a 1024-row main chunk (128 partitions x 8 rows, q-major) and a
64-row tail (64 partitions x 1 row). The multiply is split across two
engines per disparity: DVE does q 0..3 plus the tail row, Pool (gpsimd)
does q 4..7, so compute (~48+44 us split) stays under the ~57 us store
stream. Outputs land in two packed DRAM tensors:
 - out_big [128, 83136]: partition p holds, for each d, the 8*(W-d)
   products of its 8 rows (column offset 8*cum[d]).
 - out_tail [64, 10392]: partition p holds, for each d, the W-d products
   of tail row p (column offset cum[d]).
Big stores issue on the ACT HWDGE ring, loads + tail stores on SP.
"""

import os

import numpy as np

import concourse.bacc as bacc
import concourse.mybir as mybir
from concourse.bass_utils import run_bass_kernel_spmd
from concourse.mybir import AluOpType
from concourse.tile import TileContext

N, C, H, W = 2, 32, 136, 240
MAX_DISP = 48
NCORES = 8
R = N * C * H                   # 8704 rows total
ROWS = R // NCORES              # 1088 rows per core
TAIL = 64                       # leftover rows (1088 = 64 + 128*8)
CPP = 8                         # rows per partition in the main chunk
QA = 4                          # q rows handled by DVE (rest go to Pool)

# cum[d] = sum_{k<d} (W-k): packed column offset for disparity d.
CUM = [0] * (MAX_DISP + 1)
for _d in range(MAX_DISP):
    CUM[_d + 1] = CUM[_d] + (W - _d)
PACKED = CUM[MAX_DISP]          # 10392 valid products per row

_NC_CACHE = None
LAST_RESULTS = None  # BassKernelResults of the most recent run (for test.py)


def _build_bass():
    # Bacc (not plain Bass): its finalize() runs the compile pipeline incl.
    # generate_event_semaphores, which splits multi-sem waits that walrus
    # rejects ("Too many sync wait commands").
    nc = bacc.Bacc()
    left = nc.dram_tensor("left", [ROWS, W], mybir.dt.float16, kind="ExternalInput")
    right = nc.dram_tensor("right", [ROWS, W], mybir.dt.float16, kind="ExternalInput")
    out_big = nc.dram_tensor(
        "out_big", [128, CPP * PACKED], mybir.dt.float16, kind="ExternalOutput"
    )
    out_tail = nc.dram_tensor(
        "out_tail", [TAIL, PACKED], mybir.dt.float16, kind="ExternalOutput"
    )

    with (
        TileContext(nc) as tc,
        tc.tile_pool(name="lpool", bufs=2) as lpool,
        tc.tile_pool(name="rpool", bufs=2) as rpool,
        tc.tile_pool(name="obig", bufs=10) as obig,
        tc.tile_pool(name="otail", bufs=10) as otail,
    ):
        # Main chunk rows [64, 1088) as [128, 8*W]: partition p holds rows
        # 64 + 8p .. 64 + 8p + 7. Split into q 0..QA (DVE) / QA..8 (Pool)
        # halves so each engine's first multiply waits only on its own load.
        lbA = lpool.tile([128, QA * W], mybir.dt.float16, tag="lbigA")
        rbA = rpool.tile([128, QA * W], mybir.dt.float16, tag="rbigA")
        lbB = lpool.tile([128, (CPP - QA) * W], mybir.dt.float16, tag="lbigB")
        rbB = rpool.tile([128, (CPP - QA) * W], mybir.dt.float16, tag="rbigB")
        lt = lpool.tile([TAIL, W], mybir.dt.float16, tag="ltail")
        rt = rpool.tile([TAIL, W], mybir.dt.float16, tag="rtail")

        big3 = lambda t: t.rearrange("(p q) w -> p q w", p=128)
        nc.sync.dma_start(
            out=lbA[:].rearrange("p (q w) -> p q w", w=W),
            in_=big3(left[TAIL:ROWS, :])[:, 0:QA, :],
        )
        nc.sync.dma_start(
            out=rbA[:].rearrange("p (q w) -> p q w", w=W),
            in_=big3(right[TAIL:ROWS, :])[:, 0:QA, :],
        )
        nc.sync.dma_start(out=lt[:], in_=left[0:TAIL, :])
        nc.sync.dma_start(out=rt[:], in_=right[0:TAIL, :])
        nc.sync.dma_start(
            out=lbB[:].rearrange("p (q w) -> p q w", w=W),
            in_=big3(left[TAIL:ROWS, :])[:, QA:CPP, :],
        )
        nc.sync.dma_start(
            out=rbB[:].rearrange("p (q w) -> p q w", w=W),
            in_=big3(right[TAIL:ROWS, :])[:, QA:CPP, :],
        )

        lbAv = lbA[:].rearrange("p (q w) -> p q w", w=W)
        rbAv = rbA[:].rearrange("p (q w) -> p q w", w=W)
        lbBv = lbB[:].rearrange("p (q w) -> p q w", w=W)
        rbBv = rbB[:].rearrange("p (q w) -> p q w", w=W)
        for d in range(MAX_DISP):
            w = W - d
            ob = obig.tile([128, CPP * W], mybir.dt.float16)
            obv = ob[:, 0 : CPP * w].rearrange("p (q w) -> p q w", w=w)
            nc.vector.tensor_tensor(
                obv[:, 0:QA, :], lbAv[:, :, d:W], rbAv[:, :, 0:w], AluOpType.mult
            )
            nc.gpsimd.tensor_tensor(
                obv[:, QA:CPP, :], lbBv[:, :, d:W], rbBv[:, :, 0:w], AluOpType.mult
            )
            nc.scalar.dma_start(
                out=out_big[:, CPP * CUM[d] : CPP * CUM[d] + CPP * w],
                in_=ob[:, 0 : CPP * w],
            )
            ot = otail.tile([TAIL, W], mybir.dt.float16)
            nc.vector.tensor_tensor(
                ot[:, 0:w], lt[:, d:W], rt[:, 0:w], AluOpType.mult
            )
            nc.sync.dma_start(
                out=out_tail[:, CUM[d] : CUM[d] + w], in_=ot[:, 0:w]
            )
    nc.finalize()
    return nc


def kernel(left: np.ndarray, right: np.ndarray) -> np.ndarray:
    global _NC_CACHE, LAST_RESULTS
    left = np.ascontiguousarray(np.asarray(left, dtype=np.float32))
    right = np.ascontiguousarray(np.asarray(right, dtype=np.float32))
    assert left.shape == (N, C, H, W) and right.shape == (N, C, H, W)

    if _NC_CACHE is None:
        _NC_CACHE = _build_bass()
    nc = _NC_CACHE

    left_flat = np.ascontiguousarray(left.reshape(R, W).astype(np.float16))
    right_flat = np.ascontiguousarray(right.reshape(R, W).astype(np.float16))
    in_maps = [
        {
            "left": left_flat[ROWS * k : ROWS * (k + 1)],
            "right": right_flat[ROWS * k : ROWS * (k + 1)],
        }
        for k in range(NCORES)
    ]

    trace = os.environ.get("COSTVOL_TRACE", "0") == "1"
    kwargs = {}
    if os.environ.get("COSTVOL_TRACE_ALL", "0") == "1":
        kwargs["trace_cores"] = list(range(NCORES))
    res = run_bass_kernel_spmd(
        nc, in_maps, list(range(NCORES)), trace=trace, **kwargs
    )
    LAST_RESULTS = res

    flat = np.zeros((MAX_DISP, R, W), dtype=np.float32)
    for k in range(NCORES):
        big = np.asarray(res.results[k]["out_big"])
        tail = np.asarray(res.results[k]["out_tail"])
        r0 = ROWS * k
        for d in range(MAX_DISP):
            w = W - d
            c = CUM[d]
            flat[d, r0 + TAIL : r0 + ROWS, d:] = (
                big[:, CPP * c : CPP * c + CPP * w]
                .astype(np.float32)
                .reshape(128 * CPP, w)
            )
            flat[d, r0 : r0 + TAIL, d:] = tail[:, c : c + w].astype(np.float32)
    vol = flat.reshape(MAX_DISP, N, C, H, W).transpose(1, 2, 0, 3, 4)
    return np.ascontiguousarray(vol)


# revision 6
# speedup vs baseline: 1.4085x; 1.4085x over previous
"""Cost-volume kernel for Trainium2 (Bass/Tile), 8-core SPMD.

volume[n, c, d, h, w] = left[n,c,h,w] * right[n,c,h,w-d]  (0 where w < d)

Sharding: rows (flattened n,c,h = 8704) split as 1088 per core; every core
computes ALL 48 disparities for its rows (shift is along W, so row sharding
needs no halo and inputs are read once).

The kernel is HBM-store bound, so the store stream is minimized two ways:
 - fp16 output (harness gate is rel_err < 2e-2; fp16 product error ~7e-4).
 - packed layout: for disparity d only the ~(W-d) valid products
   packed[d][r, j] = left[r, d+j] * right[r, j] are stored; the host
   scatters them into a zero-filled full volume.

All multiplies run on DVE (gpsimd tensor_tensor measured ~3x slower and the
ACT engine has no two-tensor op). DVE does ~0.52 ns/elem (2x_1p fp16 mode)
plus ~350 ns fixed cost per instruction, so disparities are processed in
GROUPS of 4 per instruction using a 4-D access pattern whose group dim has
stride +1 on the left operand (one extra shift per group member) and
stride 0 (broadcast) on the right operand. Group blocks share a uniform
width W-g, so members i>0 carry (d-g) junk columns that the host ignores.

Main chunk: rows [64,1088) as [128 partitions x 8 rows]; per-partition
lines are DRAM-contiguous so every load/store is a >=3 KB-per-partition
single DMA. Disparities 0..3 are emitted per-d (even width) so the store
stream starts after ~1.4 us; d 4..47 go in 11 groups of 4. The 64-row
tail is ONE flat [64, 48, 240] multiply + one store. Big stores ride the
ACT HWDGE ring; loads and the tail store ride SP.
"""

import os

import numpy as np

import bass_rust
import concourse.bacc as bacc
import concourse.mybir as mybir
from concourse.bass_utils import run_bass_kernel_spmd
from concourse.mybir import AluOpType
from concourse.tile import TileContext

N, C, H, W = 2, 32, 136, 240
MAX_DISP = 48
NCORES = 8
R = N * C * H                   # 8704 rows total
ROWS = R // NCORES              # 1088 rows per core
TAIL = 64                       # leftover rows (1088 = 64 + 128*8)
CPP = 8                         # rows per partition in the main chunk
G = 4                           # disparities per grouped DVE instruction
NSOLO = 4                       # leading disparities emitted per-d
LBW = CPP * W + 8               # lb tile width (pad: group reads to 1919+3)
LTW = W + MAX_DISP              # lt tile width (tail reads to 286)

# Even-rounded block width for the per-d leading blocks (alignment-safe).
BW = [W - d + ((W - d) & 1) for d in range(NSOLO)]

# out_big per-partition column offsets: NSOLO per-d blocks [8, BW[d]] then
# 11 groups [G, 8, W-g].
XB = {}
_col = 0
for _d in range(NSOLO):
    XB[_d] = _col
    _col += CPP * BW[_d]
for _g in range(NSOLO, MAX_DISP, G):
    XB[_g] = _col
    _col += G * CPP * (W - _g)
XBTOT = _col                    # 83680

_NC_CACHE = None
LAST_RESULTS = None  # BassKernelResults of the most recent run (for test.py)


def _build_bass():
    # Bacc (not plain Bass): its finalize() runs the compile pipeline incl.
    # generate_event_semaphores, which splits multi-sem waits that walrus
    # rejects ("Too many sync wait commands").
    nc = bacc.Bacc()
    left = nc.dram_tensor("left", [ROWS, W], mybir.dt.float16, kind="ExternalInput")
    right = nc.dram_tensor("right", [ROWS, W], mybir.dt.float16, kind="ExternalInput")
    out_big = nc.dram_tensor(
        "out_big", [128, XBTOT], mybir.dt.float16, kind="ExternalOutput"
    )
    out_tail = nc.dram_tensor(
        "out_tail", [TAIL, MAX_DISP * W], mybir.dt.float16, kind="ExternalOutput"
    )

    with (
        TileContext(nc) as tc,
        tc.tile_pool(name="lpool", bufs=1) as lpool,
        tc.tile_pool(name="rpool", bufs=1) as rpool,
        tc.tile_pool(name="osolo", bufs=4) as osolo,
        tc.tile_pool(name="ogrp", bufs=3) as ogrp,
        tc.tile_pool(name="otail", bufs=1) as otail,
    ):
        lb = lpool.tile([128, LBW], mybir.dt.float16, tag="lbig")
        rb = rpool.tile([128, CPP * W], mybir.dt.float16, tag="rbig")
        lt = lpool.tile([TAIL, LTW], mybir.dt.float16, tag="ltail")
        rt = rpool.tile([TAIL, W], mybir.dt.float16, tag="rtail")

        nc.sync.dma_start(
            out=lb[:, 0 : CPP * W],
            in_=left[TAIL:ROWS, :].rearrange("(p q) w -> p (q w)", p=128),
        )
        nc.sync.dma_start(
            out=rb[:],
            in_=right[TAIL:ROWS, :].rearrange("(p q) w -> p (q w)", p=128),
        )
        nc.sync.dma_start(out=lt[:, 0:W], in_=left[0:TAIL, :])
        nc.sync.dma_start(out=rt[:], in_=right[0:TAIL, :])

        lbv = lb[:, 0 : CPP * W].rearrange("p (q w) -> p q w", w=W)
        rbv = rb[:].rearrange("p (q w) -> p q w", w=W)
        lb_ap = lb[:]

        # Leading disparities per-d: store stream starts after one ~1 us op.
        for d in range(NSOLO):
            bw = BW[d]
            ob = osolo.tile([128, CPP * W], mybir.dt.float16)
            in0 = bass_rust.AP(
                lb_ap.tensor,
                lb_ap.offset + d,
                [[LBW, 128], [W, CPP], [1, bw]],
            )
            nc.vector.tensor_tensor(
                ob[:, 0 : CPP * bw].rearrange("p (q w) -> p q w", w=bw),
                in0,
                rbv[:, :, 0:bw],
                AluOpType.mult,
            )
            nc.scalar.dma_start(
                out=out_big[:, XB[d] : XB[d] + CPP * bw], in_=ob[:, 0 : CPP * bw]
            )

        # Grouped disparities: one 4-D instruction per 4 d's. Left operand
        # group dim strides +1 (shift), right operand broadcasts.
        for g in range(NSOLO, MAX_DISP, G):
            wg = W - g
            ob = ogrp.tile([128, G * CPP * (W - NSOLO)], mybir.dt.float16)
            in0 = bass_rust.AP(
                lb_ap.tensor,
                lb_ap.offset + g,
                [[LBW, 128], [1, G], [W, CPP], [1, wg]],
            )
            in1 = rbv[:, :, 0:wg].unsqueeze(1).broadcast_to([128, G, CPP, wg])
            nc.vector.tensor_tensor(
                ob[:, 0 : G * CPP * wg].rearrange(
                    "p (i q w) -> p i q w", i=G, q=CPP
                ),
                in0,
                in1,
                AluOpType.mult,
            )
            nc.scalar.dma_start(
                out=out_big[:, XB[g] : XB[g] + G * CPP * wg],
                in_=ob[:, 0 : G * CPP * wg],
            )
            if g == 2 * G + NSOLO:
                # Tail: one flat [64, 48, 240] multiply + one store, placed
                # here so the big-store backlog covers its ~6 us of DVE time.
                ot = otail.tile([TAIL, MAX_DISP * W], mybir.dt.float16)
                t_in0 = bass_rust.AP(
                    lt[:].tensor,
                    lt[:].offset,
                    [[LTW, TAIL], [1, MAX_DISP], [1, W]],
                )
                t_in1 = rt[:].unsqueeze(1).broadcast_to([TAIL, MAX_DISP, W])
                nc.vector.tensor_tensor(
                    ot[:].rearrange("p (i w) -> p i w", w=W),
                    t_in0,
                    t_in1,
                    AluOpType.mult,
                )
                nc.sync.dma_start(out=out_tail[:, :], in_=ot[:])
    nc.finalize()
    return nc


def kernel(left: np.ndarray, right: np.ndarray) -> np.ndarray:
    global _NC_CACHE, LAST_RESULTS
    left = np.ascontiguousarray(np.asarray(left, dtype=np.float32))
    right = np.ascontiguousarray(np.asarray(right, dtype=np.float32))
    assert left.shape == (N, C, H, W) and right.shape == (N, C, H, W)

    if _NC_CACHE is None:
        _NC_CACHE = _build_bass()
    nc = _NC_CACHE

    left_flat = np.ascontiguousarray(left.reshape(R, W).astype(np.float16))
    right_flat = np.ascontiguousarray(right.reshape(R, W).astype(np.float16))
    in_maps = [
        {
            "left": left_flat[ROWS * k : ROWS * (k + 1)],
            "right": right_flat[ROWS * k : ROWS * (k + 1)],
        }
        for k in range(NCORES)
    ]

    trace = os.environ.get("COSTVOL_TRACE", "0") == "1"
    kwargs = {}
    if os.environ.get("COSTVOL_TRACE_ALL", "0") == "1":
        kwargs["trace_cores"] = list(range(NCORES))
    res = run_bass_kernel_spmd(
        nc, in_maps, list(range(NCORES)), trace=trace, **kwargs
    )
    LAST_RESULTS = res

    flat = np.zeros((MAX_DISP, R, W), dtype=np.float32)
    for k in range(NCORES):
        big = np.asarray(res.results[k]["out_big"])
        tail = np.asarray(res.results[k]["out_tail"]).reshape(TAIL, MAX_DISP, W)
        r0 = ROWS * k
        for d in range(NSOLO):
            w = W - d
            blk = big[:, XB[d] : XB[d] + CPP * BW[d]].reshape(128, CPP, BW[d])
            flat[d, r0 + TAIL : r0 + ROWS, d:] = (
                blk[:, :, :w].astype(np.float32).reshape(128 * CPP, w)
            )
        for g in range(NSOLO, MAX_DISP, G):
            wg = W - g
            blk = big[:, XB[g] : XB[g] + G * CPP * wg].reshape(128, G, CPP, wg)
            for i in range(G):
                d = g + i
                w = W - d
                flat[d, r0 + TAIL : r0 + ROWS, d:] = (
                    blk[:, i, :, :w].astype(np.float32).reshape(128 * CPP, w)
                )
        for d in range(MAX_DISP):
            flat[d, r0 : r0 + TAIL, d:] = tail[:, d, : W - d].astype(np.float32)
    vol = flat.reshape(MAX_DISP, N, C, H, W).transpose(1, 2, 0, 3, 4)
    return np.ascontiguousarray(vol)
